# revision 2
# baseline (speedup 1.0000x reference)
"""Trainium2 Bass kernel for a spatial self-attention block.

Reference computation (per batch element b):
    q = w1 @ x + b1   [32, HW]      (1x1 conv == channel-wise linear)
    k = w2 @ x + b2   [32, HW]
    v = w3 @ x + b3   [256, HW]
    e[i, j] = sum_c q[c, i] k[c, j]
    attn = softmax(e, axis=j)
    out[c, i] = sum_j v[c, j] attn[i, j] + x[c, i]

Sharding: batch (8) across the 8 NeuronCores, one image per core.

v2 design (vs the bf16 v1 at ~182us):
  * All big matmuls run fp8(e4m3) with MatmulPerfMode.DoubleRow (K=256 per
    instruction, 2x bf16 FLOP rate): the AV accumulation (the dominant PE
    cost), the fused [w1;w2] qk projection, and the v projection.
  * eT stays bf16 in "pack" mode (4 concurrent K=32 strip matmuls via
    tile_position) - it is PSUM-column-rate bound, fp8 cannot help it.
  * softmax exp is split across two engines per 16 slots/sweep:
      - ACT: real Exp activation, fp8e4 output directly (no extra cast)
      - DVE: Schraudolph fast-exp: one tensor_scalar computing
        round(8*log2e*e + (56-0.45)) into the pt tile viewed as uint8;
        bit-cast as e4m3 that IS 2^y with a piecewise-linear mantissa
        (~3% weighted rms vs true exp - irrelevant here, see below)
  * softmax normalization + residual fused in one DVE scalar_tensor_tensor:
    out = (po * recip(denom)) + xT, bf16 in/out.
  * Residual xT is bf16 and the output is bf16 (halves DMA); error budget:
    the attention output is ~170x smaller in norm than the residual x, so
    even ~5% attention-path error contributes only ~3e-4 to the final
    relative error (gate: 2e-2). Measured end-to-end rel err ~1e-3,
    dominated by the bf16 residual quantization.
  * Weights are pre-scaled by 64 on the host (power of two) so fp8
    quantization of the 0.02-scale weights stays in the normal range; the
    1/64 is folded into the (already required) PSUM->SBUF evacuations.
"""

import numpy as np
import ml_dtypes

B, C, H, W = 8, 256, 64, 64
HW = H * W          # 4096
CQK = C // 8        # 32
NCORES = 8
NJ = HW // 128      # 32 key tiles
ICH = 8             # query-dim chunks (pipelined A->B)
CHUNK = HW // ICH   # 512 queries per chunk
NQ = NJ // 4        # 8 quads of key tiles per chunk
VSTRIDE = 260       # vT free-dim stride per j-tile (257 used, 260 for align)
WSCALE = 64.0       # host-side pow2 weight prescale for fp8 range

# Schraudolph fast-exp constants (e4m3 bit pattern): bits = A8*e + B8
A8 = 8.0 * 1.4426950408889634   # 8 * log2(e)
B8 = 56.0 - 0.45                # (bias 7)*8 minus tuned PWL offset
# which of the 16 exp slots per sweep go to DVE (rest go to ACT)
DVE_SLOTS = (1, 3, 5, 8, 10, 12, 14)

_cache: dict = {}


def _build_program():
    import concourse.bacc as bacc
    import concourse.mybir as mybir
    import concourse.tile as tile

    f32 = mybir.dt.float32
    bf16 = mybir.dt.bfloat16
    fp8 = mybir.dt.float8e4
    u8 = mybir.dt.uint8
    Exp = mybir.ActivationFunctionType.Exp
    Identity = mybir.ActivationFunctionType.Identity
    DR = mybir.MatmulPerfMode.DoubleRow
    MUL = mybir.AluOpType.mult
    ADD = mybir.AluOpType.add

    nc = bacc.Bacc(None)
    x8_d = nc.dram_tensor("x8", [128, 2, HW], fp8, kind="ExternalInput")
    xt_d = nc.dram_tensor("xt", [HW, C], bf16, kind="ExternalInput")
    w12_d = nc.dram_tensor("w12t", [128, 2, 2 * CQK], fp8, kind="ExternalInput")
    w3_d = nc.dram_tensor("w3t", [128, 2, C], fp8, kind="ExternalInput")
    b12_d = nc.dram_tensor("b12", [2 * CQK, 1], f32, kind="ExternalInput")
    outt_d = nc.dram_tensor("outt", [HW, C], bf16, kind="ExternalOutput")

    with tile.TileContext(nc) as tc:
        with (
            tc.tile_pool(name="const", bufs=1) as cpool,
            tc.tile_pool(name="xin", bufs=1) as xpool,
            tc.tile_pool(name="qk", bufs=1) as qkpool,
            tc.tile_pool(name="pt", bufs=36) as ptpool,
            tc.tile_pool(name="io", bufs=3) as iopool,
        ):
            # ---- constants / weights ----
            w12t = cpool.tile([128, 2, 2 * CQK], fp8, tag="w12t", name="w12t")
            w3t = cpool.tile([128, 2, C], fp8, tag="w3t", name="w3t")
            b12 = cpool.tile([2 * CQK, 1], f32, tag="b12", name="b12")
            nc.sync.dma_start(w12t[:], w12_d[:])
            nc.sync.dma_start(w3t[:], w3_d[:])
            nc.sync.dma_start(b12[:], b12_d[:])

            # PE clock warmup (HAM activity monitor -> 2.4 GHz)
            warm = cpool.tile([128, 512], bf16, tag="warm", name="warm")
            nc.vector.memset(warm[:], 0.0)
            wpool = tc.tile_pool(name="psumw", bufs=1, space="PSUM")

            # x in fp8, column-chunked so qk matmuls start early
            x8 = xpool.tile([128, 2, HW], fp8, tag="x8", name="x8")
            for g in range(4):
                cs = slice(g * 1024, (g + 1) * 1024)
                nc.sync.dma_start(x8[:, :, cs], x8_d[:, :, cs])

            # q and k live replicated 4x along the partition axis (copies at
            # base partitions 0/32/64/96) so the eT pack matmuls can use all
            # 128 PE rows. vt[j, c] holds v (plus a ones column at 256 that
            # makes the softmax denominator fall out of the AV accumulation).
            q_sb = qkpool.tile([128, HW], bf16, tag="q", name="q")
            k_sb = qkpool.tile([128, HW], bf16, tag="k", name="k")
            vt = qkpool.tile([128, NJ, VSTRIDE], fp8, tag="vt", name="vt")
            nc.vector.memset(vt[:, :, 256:257], 1.0)

            with wpool as wp, \
                 tc.tile_pool(name="psumqk", bufs=2, space="PSUM") as qkps, \
                 tc.tile_pool(name="psumv", bufs=2, space="PSUM") as vps:
                wacc = wp.tile([128, 512], f32, tag="w", name="wacc")

                def warm_mm(n=1):
                    for _ in range(n):
                        nc.tensor.matmul(wacc[:], warm[:, 0:128], warm[:],
                                         start=True, stop=True)

                warm_mm(16)

                # ---- qk + v projections, 512-col chunks ----
                for g in range(ICH):
                    cs = slice(g * CHUNK, (g + 1) * CHUNK)
                    acc = qkps.tile([2 * CQK, CHUNK], f32, tag="qk", name="qk")
                    nc.tensor.matmul(acc[:], w12t[:], x8[:, :, cs],
                                     start=True, stop=True, perf_mode=DR)
                    stage = iopool.tile([2 * CQK, CHUNK], bf16, tag="st",
                                        name="st")
                    # evac alternates ACT/DVE; applies 1/WSCALE and bias
                    if g % 2 == 0:
                        nc.scalar.activation(stage[:], acc[:], Identity,
                                             bias=b12[:], scale=1.0 / WSCALE)
                    else:
                        nc.vector.tensor_scalar(stage[:], acc[:],
                                                1.0 / WSCALE, b12[:], MUL, ADD)
                    for t in range(4):
                        nc.sync.dma_start(q_sb[t * CQK:(t + 1) * CQK, cs],
                                          stage[0:CQK, :])
                        nc.sync.dma_start(k_sb[t * CQK:(t + 1) * CQK, cs],
                                          stage[CQK:2 * CQK, :])
                    # v for the 4 j-tiles of this chunk: one psum tile, one
                    # batched evacuation
                    vacc = vps.tile([128, 4, C], f32, tag="v", name="v")
                    for jj in range(4):
                        j = 4 * g + jj
                        nc.tensor.matmul(
                            vacc[:, jj, :],
                            x8[:, :, j * 128:(j + 1) * 128], w3t[:],
                            start=True, stop=True, perf_mode=DR)
                    j0 = 4 * g
                    if g % 2 == 0:
                        nc.vector.tensor_scalar(
                            vt[:, j0:j0 + 4, 0:C], vacc[:], 1.0 / WSCALE,
                            None, MUL)
                    else:
                        nc.scalar.activation(vt[:, j0:j0 + 4, 0:C], vacc[:],
                                             mybir.ActivationFunctionType.Copy,
                                             scale=1.0 / WSCALE)
                    warm_mm(1)

            # ---- attention: A (eT quad + exp) and B (out accumulation) ----
            # A unit (s, u, p): 2 key tiles against query chunk s: 2 pack
            # matmuls (32-row strips at tile_position rows) -> one 2-bank
            # PSUM tile -> one N=1024 exp (ACT or DVE-Schraudolph) -> pt fp8.
            # B unit (s, u): i-tile u//2 of chunk s, key half u%2: 8 fp8
            # DoubleRow matmuls accumulating [128, 257] (v | denominator).
            pt_handles = [[None] * (2 * NQ) for _ in range(ICH)]
            with tc.tile_pool(name="psume", bufs=3, space="PSUM") as epool, \
                 tc.tile_pool(name="psumo", bufs=2, space="PSUM") as opool:
                po = None
                xt_t = None
                for s in range(ICH + 1):
                    for u in range(NQ):
                        if s < ICH:
                            for p in range(2):
                                slot = 2 * u + p
                                ep = epool.tile([128, 2, CHUNK], f32, tag="e",
                                                name="e")
                                if s == 0:
                                    # keep PE busy/warm while exps drain
                                    nc.tensor.matmul(ep[:, 0, :],
                                                     warm[:, 0:128], warm[:],
                                                     start=True, stop=True)
                                for i in range(2):
                                    t = 2 * p + i
                                    jt = 4 * u + t
                                    nc.tensor.matmul(
                                        ep[:, i, :],
                                        k_sb[t * CQK:(t + 1) * CQK,
                                             jt * 128:(jt + 1) * 128],
                                        q_sb[t * CQK:(t + 1) * CQK,
                                             s * CHUNK:(s + 1) * CHUNK],
                                        start=True, stop=True,
                                        tile_position=(t * CQK, 0))
                                pt = ptpool.tile([128, 2, CHUNK], fp8,
                                                 tag="pt", name="pt")
                                if slot in DVE_SLOTS:
                                    nc.vector.tensor_scalar(
                                        pt.bitcast(u8)[:], ep[:], A8, B8,
                                        MUL, ADD)
                                else:
                                    nc.scalar.activation(pt[:], ep[:], Exp)
                                pt_handles[s][slot] = pt
                        if s >= 1:
                            bs = s - 1
                            it, half = u // 2, u % 2
                            i0 = bs * CHUNK + it * 128
                            if half == 0:
                                po = opool.tile([128, 257], f32, tag="o",
                                                name="o")
                                xt_t = iopool.tile([128, C], bf16, tag="xt",
                                                   name="xt")
                                nc.sync.dma_start(xt_t[:], xt_d[i0:i0 + 128, :])
                            for jj in range(8 * half, 8 * half + 8):
                                nc.tensor.matmul(
                                    po[:],
                                    pt_handles[bs][jj][:, :,
                                                       it * 128:(it + 1) * 128],
                                    vt[:, 2 * jj:2 * jj + 2, 0:257],
                                    start=(jj == 0), stop=(jj == 2 * NQ - 1),
                                    perf_mode=DR)
                            if half == 1:
                                r = iopool.tile([128, 1], f32, tag="r",
                                                name="r")
                                nc.vector.reciprocal(r[:], po[:, 256:257])
                                ot = iopool.tile([128, C], bf16, tag="ot",
                                                 name="ot")
                                nc.vector.scalar_tensor_tensor(
                                    ot[:], po[:, 0:256], r[:], xt_t[:],
                                    MUL, ADD)
                                nc.sync.dma_start(outt_d[i0:i0 + 128, :], ot[:])

    nc.compile()
    return nc


def _get_program():
    if "nc" not in _cache:
        _cache["nc"] = _build_program()
    return _cache["nc"]


def _in_maps(inputs: dict) -> list:
    bf = ml_dtypes.bfloat16
    e4 = ml_dtypes.float8_e4m3
    x = np.asarray(inputs["x"], np.float32)
    w1 = np.asarray(inputs["w1"], np.float32)
    w2 = np.asarray(inputs["w2"], np.float32)
    w3 = np.asarray(inputs["w3"], np.float32)
    b1 = np.asarray(inputs["b1"], np.float32)
    b2 = np.asarray(inputs["b2"], np.float32)
    b3 = np.asarray(inputs["b3"], np.float32)
    w12 = np.concatenate([w1, w2], axis=0) * WSCALE          # [64, 256]
    w12t8 = np.ascontiguousarray(
        w12.T.reshape(2, 128, 2 * CQK).transpose(1, 0, 2)).astype(e4)
    w3t8 = np.ascontiguousarray(
        (w3 * WSCALE).T.reshape(2, 128, C).transpose(1, 0, 2)).astype(e4)
    b12 = np.concatenate([b1, b2])[:, None].astype(np.float32)
    maps = []
    for b in range(B):
        xb = x[b].reshape(C, HW)
        x8 = np.ascontiguousarray(
            xb.reshape(2, 128, HW).transpose(1, 0, 2)).astype(e4)
        maps.append({
            "x8": x8,
            "xt": (np.ascontiguousarray(xb.T) + b3[None, :]).astype(bf),
            "w12t": w12t8, "w3t": w3t8, "b12": b12,
        })
    return maps


def kernel(**inputs) -> np.ndarray:
    from concourse.bass_utils import run_bass_kernel_spmd

    nc = _get_program()
    res = run_bass_kernel_spmd(nc, _in_maps(inputs), list(range(NCORES)))
    out = np.empty((B, C, H, W), np.float32)
    for b in range(B):
        out[b] = res.results[b]["outt"].astype(np.float32).T.reshape(C, H, W)
    return out


# revision 6
# speedup vs baseline: 1.1447x; 1.1447x over previous
"""Trainium2 Bass kernel for a spatial self-attention block.

Reference computation (per batch element b):
    q = w1 @ x + b1   [32, HW]      (1x1 conv == channel-wise linear)
    k = w2 @ x + b2   [32, HW]
    v = w3 @ x + b3   [256, HW]
    e[i, j] = sum_c q[c, i] k[c, j]
    attn = softmax(e, axis=j)
    out[c, i] = sum_j v[c, j] attn[i, j] + x[c, i]

Sharding: batch (8) across the 8 NeuronCores, one image per core.

v3 design (vs the bf16 v1 at ~182us):
  * All big matmuls run fp8(e4m3) with MatmulPerfMode.DoubleRow (K=256 per
    instruction, 2x bf16 FLOP rate): the AV accumulation (the dominant PE
    cost), the q/k projections, and the v projection.
  * q and k come out of the PE already replicated 4x along the partition
    axis (the projection weights have their 32 output columns tiled 4x to
    fill all 128 PE columns), so the eT pack matmuls need NO SBUF
    replication DMAs -- v2's 64 serialized copies stalled the ramp.
  * eT stays bf16 in "pack" mode (4 concurrent K=32 strip matmuls via
    tile_position) - it is PSUM-column-rate bound, fp8 cannot help it.
  * sweep-0 attention (eT quads + exp) is merged into the projection phase
    so the PE never idles long enough for the HAM activity monitor to
    down-clock it (v2 lost ~25us to a 1.2GHz window).
  * softmax exp is split across two engines per 16 slots/sweep:
      - ACT: real Exp activation, fp8e4 output directly
      - DVE: Schraudolph fast-exp: one tensor_scalar computing
        round(8*log2e*e + (56-0.45)) into the pt tile viewed as uint8;
        bit-cast as e4m3 that IS 2^y with a piecewise-linear mantissa
        (~3% weighted rms vs true exp - irrelevant here, see below)
  * softmax normalization + residual fused in one DVE scalar_tensor_tensor:
    out = (po * recip(denom)) + xT, bf16 in/out.
  * Residual xT is bf16 and the output is bf16 (halves DMA); error budget:
    the attention output is ~170x smaller in norm than the residual x, so
    even ~5% attention-path error contributes only ~3e-4 to the final
    relative error (gate: 2e-2). Measured end-to-end rel err ~2e-3,
    dominated by the bf16 residual quantization.
  * Weights are pre-scaled by 64 on the host (power of two) so fp8
    quantization of the 0.02-scale weights stays in the normal range; the
    1/64 is folded into the (already required) PSUM->SBUF evacuations.
  * All PSUM flows through one pool pair (ep: 3x2 banks, po: 2x1 banks,
    8 banks total): projections and PE-warm matmuls borrow ep tiles.
"""

import numpy as np
import ml_dtypes

B, C, H, W = 8, 256, 64, 64
HW = H * W          # 4096
CQK = C // 8        # 32
NCORES = 8
NJ = HW // 128      # 32 key tiles
ICH = 8             # query-dim chunks (pipelined A->B)
CHUNK = HW // ICH   # 512 queries per chunk
NQ = NJ // 4        # 8 quads of key tiles per chunk
VSTRIDE = 260       # vT free-dim stride per j-tile (257 used, 260 for align)
WSCALE = 64.0       # host-side pow2 weight prescale for fp8 range

# Schraudolph fast-exp constants (e4m3 bit pattern): bits = A8*e + B8
A8 = 8.0 * 1.4426950408889634   # 8 * log2(e)
B8 = 56.0 - 0.45                # (bias 7)*8 minus tuned PWL offset
# which of the 16 exp slots per sweep go to DVE (rest go to ACT)
DVE_SLOTS = (1, 3, 5, 8, 10, 12, 14)

_cache: dict = {}


def _build_program():
    import concourse.bacc as bacc
    import concourse.mybir as mybir
    import concourse.tile as tile

    f32 = mybir.dt.float32
    bf16 = mybir.dt.bfloat16
    fp8 = mybir.dt.float8e4
    u8 = mybir.dt.uint8
    Exp = mybir.ActivationFunctionType.Exp
    Identity = mybir.ActivationFunctionType.Identity
    Copy = mybir.ActivationFunctionType.Copy
    DR = mybir.MatmulPerfMode.DoubleRow
    MUL = mybir.AluOpType.mult
    ADD = mybir.AluOpType.add

    nc = bacc.Bacc(None)
    x8_d = nc.dram_tensor("x8", [128, 2, HW], fp8, kind="ExternalInput")
    xt_d = nc.dram_tensor("xt", [HW, C], bf16, kind="ExternalInput")
    w1_d = nc.dram_tensor("w1t4", [128, 2, 128], fp8, kind="ExternalInput")
    w2_d = nc.dram_tensor("w2t4", [128, 2, 128], fp8, kind="ExternalInput")
    w3_d = nc.dram_tensor("w3t", [128, 2, C], fp8, kind="ExternalInput")
    b1_d = nc.dram_tensor("b1r4", [128, 1], f32, kind="ExternalInput")
    b2_d = nc.dram_tensor("b2r4", [128, 1], f32, kind="ExternalInput")
    outt_d = nc.dram_tensor("outt", [HW, C], bf16, kind="ExternalOutput")

    with tile.TileContext(nc) as tc:
        with (
            tc.tile_pool(name="const", bufs=1) as cpool,
            tc.tile_pool(name="xin", bufs=1) as xpool,
            tc.tile_pool(name="qk", bufs=1) as qkpool,
            tc.tile_pool(name="pt", bufs=36) as ptpool,
            tc.tile_pool(name="io", bufs=3) as iopool,
            tc.tile_pool(name="psume", bufs=3, space="PSUM") as epool,
            tc.tile_pool(name="psumo", bufs=2, space="PSUM") as opool,
        ):
            # ---- constants / weights ----
            w1t4 = cpool.tile([128, 2, 128], fp8, tag="w1t4", name="w1t4")
            w2t4 = cpool.tile([128, 2, 128], fp8, tag="w2t4", name="w2t4")
            w3t = cpool.tile([128, 2, C], fp8, tag="w3t", name="w3t")
            b1r4 = cpool.tile([128, 1], f32, tag="b1r4", name="b1r4")
            b2r4 = cpool.tile([128, 1], f32, tag="b2r4", name="b2r4")
            nc.sync.dma_start(w1t4[:], w1_d[:])
            nc.sync.dma_start(w2t4[:], w2_d[:])
            nc.sync.dma_start(w3t[:], w3_d[:])
            nc.sync.dma_start(b1r4[:], b1_d[:])
            nc.sync.dma_start(b2r4[:], b2_d[:])

            warm = cpool.tile([128, 512], bf16, tag="warm", name="warm")
            nc.vector.memset(warm[:], 0.0)

            x8 = xpool.tile([128, 2, HW], fp8, tag="x8", name="x8")
            for g in range(8):
                cs = slice(g * CHUNK, (g + 1) * CHUNK)
                nc.sync.dma_start(x8[:, :, cs], x8_d[:, :, cs])

            # q and k live replicated 4x along the partition axis (copies at
            # base partitions 0/32/64/96) so the eT pack matmuls can use all
            # 128 PE rows. vt[j, c] holds v (plus a ones column at 256 that
            # makes the softmax denominator fall out of the AV accumulation).
            q_sb = qkpool.tile([128, HW], bf16, tag="q", name="q")
            k_sb = qkpool.tile([128, HW], bf16, tag="k", name="k")
            vt = qkpool.tile([128, NJ // 2, 2, VSTRIDE], fp8, tag="vt",
                             name="vt")
            nc.vector.memset(vt[:, :, :, 256:257], 1.0)

            def warm_mm(dst, n=1):
                # dummy full-array matmul into a psum region that a real
                # matmul overwrites later (start=True resets it); trips the
                # HAM activity monitor so the PE clock ramps to 2.4 GHz
                for _ in range(n):
                    nc.tensor.matmul(dst, warm[:, 0:128], warm[:],
                                     start=True, stop=True)

            wep = epool.tile([128, 2, CHUNK], f32, tag="e", name="e")
            warm_mm(wep[:, 0, :], 8)

            # ---- phase 1: projections + sweep-0 attention, per 512-col
            # chunk g: q/k/v projections (fp8 DR) -> evacs, then the sweep-0
            # eT quad for key-chunk g + its two exps. PE stays dense.
            pt_handles = [[None] * (2 * NQ) for _ in range(ICH)]
            for g in range(ICH):
                cs = slice(g * CHUNK, (g + 1) * CHUNK)
                ep_qk = epool.tile([128, 2, CHUNK], f32, tag="e", name="e")
                warm_mm(ep_qk[:, 0, :])
                nc.tensor.matmul(ep_qk[:, 0, :], w1t4[:], x8[:, :, cs],
                                 start=True, stop=True, perf_mode=DR)
                nc.tensor.matmul(ep_qk[:, 1, :], w2t4[:], x8[:, :, cs],
                                 start=True, stop=True, perf_mode=DR)
                nc.scalar.activation(q_sb[:, cs], ep_qk[:, 0, :], Identity,
                                     bias=b1r4[:], scale=1.0 / WSCALE)
                nc.vector.tensor_scalar(k_sb[:, cs], ep_qk[:, 1, :],
                                        1.0 / WSCALE, b2r4[:], MUL, ADD)
                # v for the 4 j-tiles of this chunk: one borrowed ep tile,
                # one batched 4-j-tile evacuation
                ep_v = epool.tile([128, 2, CHUNK], f32, tag="e", name="e")
                for jj in range(4):
                    j = 4 * g + jj
                    nc.tensor.matmul(
                        ep_v[:, jj // 2,
                             (jj % 2) * 256:(jj % 2) * 256 + 256],
                        x8[:, :, j * 128:(j + 1) * 128], w3t[:],
                        start=True, stop=True, perf_mode=DR)
                # evac h covers j-tiles {4g+h, 4g+2+h} = vt[:, 2g:2g+2, h]
                for h in range(2):
                    src = ep_v[:, :, h * 256:h * 256 + 256]
                    dst = vt[:, 2 * g:2 * g + 2, h, 0:C]
                    if (g + h) % 2 == 0:
                        nc.vector.tensor_scalar(dst, src, 1.0 / WSCALE,
                                                None, MUL)
                    else:
                        nc.scalar.activation(dst, src, Copy,
                                             scale=1.0 / WSCALE)
                # sweep-0 quad for key tiles 4g..4g+3 vs query chunk 0
                for p in range(2):
                    slot = 2 * g + p
                    ep = epool.tile([128, 2, CHUNK], f32, tag="e", name="e")
                    warm_mm(ep[:, 0, :])
                    for i in range(2):
                        t = 2 * p + i
                        jt = 4 * g + t
                        nc.tensor.matmul(
                            ep[:, i, :],
                            k_sb[t * CQK:(t + 1) * CQK,
                                 jt * 128:(jt + 1) * 128],
                            q_sb[t * CQK:(t + 1) * CQK, 0:CHUNK],
                            start=True, stop=True,
                            tile_position=(t * CQK, 0))
                    pt = ptpool.tile([128, 2, CHUNK], fp8, tag="pt",
                                     name="pt")
                    if slot in DVE_SLOTS:
                        nc.vector.tensor_scalar(pt.bitcast(u8)[:], ep[:],
                                                A8, B8, MUL, ADD)
                    else:
                        nc.scalar.activation(pt[:], ep[:], Exp)
                    pt_handles[0][slot] = pt

            # ---- sweeps 1..8: A (eT quad + exp) and B (out accumulation) --
            # A unit (s, u, p): 2 key tiles vs query chunk s: 2 pack matmuls
            # (32-row strips at tile_position rows) -> one 2-bank PSUM tile
            # -> one N=1024 exp (ACT or DVE-Schraudolph) -> pt fp8.
            # B unit (s-1, u): i-tile u//2 of chunk s-1, key half u%2: 8 fp8
            # DoubleRow matmuls accumulating [128, 257] (v | denominator).
            po = None
            xt_t = None
            for s in range(1, ICH + 1):
                for u in range(NQ):
                    if s < ICH:
                        for p in range(2):
                            slot = 2 * u + p
                            ep = epool.tile([128, 2, CHUNK], f32, tag="e",
                                            name="e")
                            for i in range(2):
                                t = 2 * p + i
                                jt = 4 * u + t
                                nc.tensor.matmul(
                                    ep[:, i, :],
                                    k_sb[t * CQK:(t + 1) * CQK,
                                         jt * 128:(jt + 1) * 128],
                                    q_sb[t * CQK:(t + 1) * CQK,
                                         s * CHUNK:(s + 1) * CHUNK],
                                    start=True, stop=True,
                                    tile_position=(t * CQK, 0))
                            pt = ptpool.tile([128, 2, CHUNK], fp8,
                                             tag="pt", name="pt")
                            if slot in DVE_SLOTS:
                                nc.vector.tensor_scalar(
                                    pt.bitcast(u8)[:], ep[:], A8, B8,
                                    MUL, ADD)
                            else:
                                nc.scalar.activation(pt[:], ep[:], Exp)
                            pt_handles[s][slot] = pt
                    bs = s - 1
                    it, half = u // 2, u % 2
                    i0 = bs * CHUNK + it * 128
                    if half == 0:
                        po = opool.tile([128, 257], f32, tag="o", name="o")
                        xt_t = iopool.tile([128, C], bf16, tag="xt",
                                           name="xt")
                        nc.sync.dma_start(xt_t[:], xt_d[i0:i0 + 128, :])
                    for jj in range(8 * half, 8 * half + 8):
                        nc.tensor.matmul(
                            po[:],
                            pt_handles[bs][jj][:, :,
                                               it * 128:(it + 1) * 128],
                            vt[:, jj, :, 0:257],
                            start=(jj == 0), stop=(jj == 2 * NQ - 1),
                            perf_mode=DR)
                    if half == 1:
                        r = iopool.tile([128, 1], f32, tag="r", name="r")
                        nc.vector.reciprocal(r[:], po[:, 256:257])
                        ot = iopool.tile([128, C], bf16, tag="ot", name="ot")
                        nc.vector.scalar_tensor_tensor(
                            ot[:], po[:, 0:256], r[:], xt_t[:], MUL, ADD)
                        nc.sync.dma_start(outt_d[i0:i0 + 128, :], ot[:])

    nc.compile()
    return nc


def _get_program():
    if "nc" not in _cache:
        _cache["nc"] = _build_program()
    return _cache["nc"]


def _in_maps(inputs: dict) -> list:
    bf = ml_dtypes.bfloat16
    e4 = ml_dtypes.float8_e4m3
    x = np.asarray(inputs["x"], np.float32)
    w1 = np.asarray(inputs["w1"], np.float32)
    w2 = np.asarray(inputs["w2"], np.float32)
    w3 = np.asarray(inputs["w3"], np.float32)
    b1 = np.asarray(inputs["b1"], np.float32)
    b2 = np.asarray(inputs["b2"], np.float32)
    b3 = np.asarray(inputs["b3"], np.float32)

    def rep4(w):  # [32, 256] -> [128, 2, 128] stationary, out cols tiled 4x
        wr = np.tile(w * WSCALE, (4, 1))                     # [128, 256]
        return np.ascontiguousarray(
            wr.T.reshape(2, 128, 128).transpose(1, 0, 2)).astype(e4)

    w1t4 = rep4(w1)
    w2t4 = rep4(w2)
    w3t8 = np.ascontiguousarray(
        (w3 * WSCALE).T.reshape(2, 128, C).transpose(1, 0, 2)).astype(e4)
    b1r4 = np.tile(b1, 4)[:, None].astype(np.float32)
    b2r4 = np.tile(b2, 4)[:, None].astype(np.float32)
    maps = []
    for b in range(B):
        xb = x[b].reshape(C, HW)
        x8 = np.ascontiguousarray(
            xb.reshape(2, 128, HW).transpose(1, 0, 2)).astype(e4)
        maps.append({
            "x8": x8,
            "xt": (np.ascontiguousarray(xb.T) + b3[None, :]).astype(bf),
            "w1t4": w1t4, "w2t4": w2t4, "w3t": w3t8,
            "b1r4": b1r4, "b2r4": b2r4,
        })
    return maps


def kernel(**inputs) -> np.ndarray:
    from concourse.bass_utils import run_bass_kernel_spmd

    nc = _get_program()
    res = run_bass_kernel_spmd(nc, _in_maps(inputs), list(range(NCORES)))
    out = np.empty((B, C, H, W), np.float32)
    for b in range(B):
        out[b] = res.results[b]["outt"].astype(np.float32).T.reshape(C, H, W)
    return out
